# revision 1
# baseline (speedup 1.0000x reference)
"""Trainium2 Bass kernel for nn_CodecAttention (GQA + qk-RMSNorm + ALiBi
sliding-window attention + output projection), sharded over 8 NeuronCores as
batch(2) x sequence-chunk(4).

Per core: 512 query tokens (all 16 heads), 1024 halo tokens for K/V.
ALiBi bias + position masking are folded into the score matmul via two extra
contraction rows (rank-2 trick); softmax is the no-max-subtraction variant
(safe because the folded bias is <= 0 and exp(s_kk') ~ e^{+-5}).
Softmax denominator comes from a ones-column appended to V in the PV matmul.
"""

import math
import sys

import numpy as np

sys.path.insert(0, "/opt/trn_rl_repo")

import bass_rust
import concourse.bass as bass
import concourse.tile as tile
from concourse import mybir
from concourse.bass_utils import run_bass_kernel_spmd
from concourse.masks import make_identity
from concourse.vector_clock import ScopedClock

F32 = mybir.dt.float32
BF16 = mybir.dt.bfloat16
AF = mybir.ActivationFunctionType

DIM = 1024
N_HEADS = 16
N_KV = 4
HD = 64
WINDOW = 512
EPS = 1e-6
B, S = 2, 2048
CH = 512           # query chunk per core
HALO = 1024        # key rows per core (window + chunk)
NT_Q = CH // 128   # 4 query tiles
NT_K = HALO // 128 # 8 key tiles
PAD_POS = -1.0e5

# ---------------------------------------------------------------------------
# Workaround: walrus rejects the TileContext kernel-tail drain when it carries
# more than ~2 sem waits ("Too many sync wait commands").  Spread the
# global-clock waits across multiple SP drain instructions (1 wait each).
_MAXW = 1


def _patched_drain_and_barrier(self, tick_clock, wait_clock):
    nc = self.nc
    probe = nc.sync.drain()
    wait_clock.add_sem_waits(probe.ins, ScopedClock({None: tick_clock.global_clock}))
    si = probe.ins.sync_info
    waits = list(si.on_wait) if si is not None else []
    if len(waits) > _MAXW:
        si.on_wait = waits[:_MAXW]
        for i in range(_MAXW, len(waits), _MAXW):
            d = nc.sync.drain()
            if d.ins.sync_info is None:
                d.ins.sync_info = bass_rust.SyncInfo(
                    on_wait=list(waits[i : i + _MAXW]), on_update=[]
                )
            else:
                d.ins.sync_info.on_wait = list(waits[i : i + _MAXW])
    nc.all_engine_barrier()
    assert self.sems is not None
    popped = nc._tile_sem_poison_stack.pop()
    assert popped is self._sem_poison
    nc.clear_and_free_semaphores(list(self.sems.allocated().values()))
    nc.all_engine_barrier()


tile.TileContext._drain_and_barrier = _patched_drain_and_barrier

# Also split waits on regular instructions: walrus rejects >2 sync waits per
# instruction.  Hook the final-emission path and move excess sem-ge waits onto
# same-engine carrier nops inserted immediately before the instruction.
_SPLIT_MAXW = 1
_orig_add_instruction = tile.TileContext._add_instruction


def _patched_add_instruction(self, inst):
    si = getattr(inst, "sync_info", None)
    if si is not None and si.on_wait and len(si.on_wait) > _SPLIT_MAXW:
        waits = list(si.on_wait)
        eq = [w for w in waits if w.wait_mode != "sem-ge-imm"]
        ge = [w for w in waits if w.wait_mode == "sem-ge-imm"]
        ordered = eq + ge
        keep, extra = ordered[:_SPLIT_MAXW], ordered[_SPLIT_MAXW:]
        assert len(eq) <= _SPLIT_MAXW, "cannot split non-monotonic waits"
        si.on_wait = keep
        for i in range(0, len(extra), _SPLIT_MAXW):
            nop = mybir.InstNoOp(
                name=f"{inst.name}-wsplit{i}",
                sync_info=mybir.SyncInfo(
                    on_wait=list(extra[i : i + _SPLIT_MAXW]), on_update=[]
                ),
                bass_nofuse=True,
                engine=inst.engine,
            )
            _orig_add_instruction(self, nop)
    _orig_add_instruction(self, inst)


tile.TileContext._add_instruction = _patched_add_instruction
# ---------------------------------------------------------------------------


def _alibi_slopes(n_heads):
    r = 2.0 ** (-8.0 / n_heads)
    return np.array([r ** (i + 1) for i in range(n_heads)], dtype=np.float32)


def build_program(repeat=1, phases=(1, 3, 4)):
    nc = bass.Bass("TRN2", target_bir_lowering=False, debug=False, num_devices=8)

    xT = nc.dram_tensor("xT", [DIM, HALO], F32, kind="ExternalInput").ap()
    wqT = nc.dram_tensor("wqT", [DIM, DIM], F32, kind="ExternalInput").ap()
    wkT = nc.dram_tensor("wkT", [DIM, N_KV * HD], F32, kind="ExternalInput").ap()
    wvT = nc.dram_tensor("wvT", [DIM, N_KV * HD], F32, kind="ExternalInput").ap()
    woT = nc.dram_tensor("woT", [DIM, DIM], F32, kind="ExternalInput").ap()
    qaug = nc.dram_tensor("qaug", [N_HEADS, 2, CH], F32, kind="ExternalInput").ap()
    kaug = nc.dram_tensor("kaug", [2, HALO], F32, kind="ExternalInput").ap()
    qnw = nc.dram_tensor("qnw", [DIM], F32, kind="ExternalInput").ap()
    knw = nc.dram_tensor("knw", [N_KV * HD], F32, kind="ExternalInput").ap()
    out = nc.dram_tensor("out", [CH, DIM], F32, kind="ExternalOutput").ap()

    from contextlib import ExitStack
    with tile.TileContext(nc) as tc, ExitStack() as stack:
        # ---------------- persistent pools -------------------------------
        singles = stack.enter_context(tc.tile_pool(name="singles", bufs=1))
        ident = singles.tile([128, 128], F32)
        make_identity(nc, ident)

        qnw_sb = singles.tile([128, 8], F32)
        nc.gpsimd.dma_start(out=qnw_sb, in_=qnw.rearrange("(a b) -> b a", b=128))
        knw_sb = singles.tile([128, 2], F32)
        eps_sb = singles.tile([128, 1], F32)
        nc.vector.memset(eps_sb, EPS)
        nc.gpsimd.dma_start(out=knw_sb, in_=knw.rearrange("(a b) -> b a", b=128))

        # normalized, d-major q/k with 2 aug rows appended (partitions 0..65)
        qT_pool = stack.enter_context(tc.tile_pool(name="qT", bufs=1))
        qT = [qT_pool.tile([66, CH], F32, tag=f"qT{h}", name=f"qT{h}") for h in range(N_HEADS)]
        kT = [qT_pool.tile([66, HALO], F32, tag=f"kT{g}", name=f"kT{g}") for g in range(N_KV)]
        for h in range(N_HEADS):
            nc.gpsimd.dma_start(out=qT[h][64:66, :], in_=qaug[h, :, :])
        for g in range(N_KV):
            nc.gpsimd.dma_start(out=kT[g][64:66, :], in_=kaug[:, :])

        # token-major V (+ ones column) per key tile: [128, N_KV, HD+1]
        v_pool = stack.enter_context(tc.tile_pool(name="v", bufs=1))
        v_sb = [v_pool.tile([128, N_KV, HD + 1], BF16, tag=f"v{b}", name=f"v{b}") for b in range(NT_K)]
        for b in range(NT_K):
            nc.vector.memset(v_sb[b][:, :, HD : HD + 1], 1.0)

        # ---------------- phase 3: attention -----------------------------
        yT_pool = stack.enter_context(tc.tile_pool(name="yT", bufs=1))
        yT = [yT_pool.tile([128, CH], F32, tag=f"yT{c}", name=f"yT{c}") for c in range(8)]

        for _rep in range(repeat):
          if 1 in phases:
            # ---------------- phase 1+2: projections, norms, transposes ------
            with tc.tile_pool(name="w_in", bufs=1) as w_in, \
                 tc.tile_pool(name="x_in", bufs=16) as x_in, \
                 tc.tile_pool(name="proj_ps", bufs=2, space="PSUM") as proj_ps, \
                 tc.tile_pool(name="tr_ps", bufs=2, space="PSUM") as tr_ps, \
                 tc.tile_pool(name="ev", bufs=2) as ev, \
                 tc.tile_pool(name="scr", bufs=2) as scr:

                wq_sb = [w_in.tile([128, DIM], F32, tag=f"wq{k}", name=f"wq{k}") for k in range(8)]
                wk_sb = [w_in.tile([128, N_KV * HD], F32, tag=f"wk{k}", name=f"wk{k}") for k in range(8)]
                wv_sb = [w_in.tile([128, N_KV * HD], F32, tag=f"wv{k}", name=f"wv{k}") for k in range(8)]
                for k in range(8):
                    nc.sync.dma_start(out=wq_sb[k], in_=wqT[bass.ts(k, 128), :])
                    nc.gpsimd.dma_start(out=wk_sb[k], in_=wkT[bass.ts(k, 128), :])
                    nc.gpsimd.dma_start(out=wv_sb[k], in_=wvT[bass.ts(k, 128), :])

                for tt in range(NT_K):
                    xt = [x_in.tile([128, 128], F32, tag="xt", name="xt") for _ in range(8)]
                    for k in range(8):
                        nc.gpsimd.dma_start(
                            out=xt[k], in_=xT[bass.ts(k, 128), bass.ts(tt, 128)]
                        )
                    is_q = tt >= NT_K - NT_Q  # last 4 token tiles are the queries
                    k_ps = proj_ps.tile([128, N_KV * HD], F32, tag="k_ps")
                    v_ps = proj_ps.tile([128, N_KV * HD], F32, tag="v_ps")
                    if is_q:
                        q_ps = proj_ps.tile([128, DIM], F32, tag="q_ps", bufs=1)
                    for k in range(8):
                        st, sp = k == 0, k == 7
                        nc.tensor.matmul(k_ps, xt[k], wk_sb[k], start=st, stop=sp)
                        nc.tensor.matmul(v_ps, xt[k], wv_sb[k], start=st, stop=sp)
                        if is_q:
                            for half in range(2):
                                nc.tensor.matmul(
                                    q_ps[:, bass.ts(half, 512)],
                                    xt[k],
                                    wq_sb[k][:, bass.ts(half, 512)],
                                    start=st,
                                    stop=sp,
                                )

                    # V: copy to SBUF (strided into [128, g, 0:64])
                    nc.vector.tensor_copy(
                        v_sb[tt][:, :, 0:HD],
                        v_ps.rearrange("p (g d) -> p g d", g=N_KV),
                    )

                    # K: sum of squares -> rms -> normalize -> transpose
                    ssk = ev.tile([128, 1], F32, tag="ssk")
                    trash_k = scr.tile([128, N_KV * HD], F32, tag="trash_k")
                    nc.scalar.activation(trash_k, k_ps, AF.Square, accum_out=ssk)
                    rmsk = ev.tile([128, 1], F32, tag="rmsk")
                    nc.scalar.activation(rmsk, ssk, AF.Sqrt, scale=1.0 / (N_KV * HD), bias=eps_sb)
                    invk = ev.tile([128, 1], F32, tag="invk")
                    nc.vector.reciprocal(invk, rmsk)
                    k_n = ev.tile([128, N_KV * HD], F32, tag="k_n")
                    nc.vector.tensor_scalar_mul(k_n, k_ps, invk)
                    for c in range(2):  # two 128-dim column blocks = 2 kv heads each
                        ktp = tr_ps.tile([128, 128], F32, tag="tp", name="ktp")
                        nc.tensor.transpose(ktp, k_n[:, bass.ts(c, 128)], ident)
                        for s in range(2):
                            g = 2 * c + s
                            nc.vector.tensor_scalar_mul(
                                kT[g][0:HD, bass.ts(tt, 128)],
                                ktp[bass.ts(s, HD), :],
                                knw_sb[bass.ts(s, HD), c : c + 1],
                            )

                    if is_q:
                        t = tt - (NT_K - NT_Q)
                        ssq = ev.tile([128, 1], F32, tag="ssq")
                        trash_q = scr.tile([128, DIM], F32, tag="trash_q")
                        nc.scalar.activation(trash_q, q_ps, AF.Square, accum_out=ssq)
                        rmsq = ev.tile([128, 1], F32, tag="rmsq")
                        nc.scalar.activation(rmsq, ssq, AF.Sqrt, scale=1.0 / DIM, bias=eps_sb)
                        invq = ev.tile([128, 1], F32, tag="invq")
                        nc.vector.reciprocal(invq, rmsq)
                        q_n = ev.tile([128, DIM], F32, tag="q_n")
                        nc.vector.tensor_scalar_mul(q_n, q_ps, invq)
                        for c in range(8):  # 128-dim blocks = 2 heads each
                            qtp = tr_ps.tile([128, 128], F32, tag="tp", name="qtp")
                            nc.tensor.transpose(qtp, q_n[:, bass.ts(c, 128)], ident)
                            for s in range(2):
                                h = 2 * c + s
                                nc.vector.tensor_scalar_mul(
                                    qT[h][0:HD, bass.ts(t, 128)],
                                    qtp[bass.ts(s, HD), :],
                                    qnw_sb[bass.ts(s, HD), c : c + 1],
                                )

          if 3 in phases:
            # ---------------- phase 3: attention -----------------------------

            with tc.tile_pool(name="s_ps", bufs=2, space="PSUM") as s_ps_pool, \
                 tc.tile_pool(name="y_ps", bufs=4, space="PSUM") as y_ps_pool, \
                 tc.tile_pool(name="yt_ps", bufs=1, space="PSUM") as yt_ps_pool, \
                 tc.tile_pool(name="probs", bufs=3) as probs_pool, \
                 tc.tile_pool(name="yev", bufs=2) as yev:

                for h in range(N_HEADS):
                    g = h // 4
                    y_ps = [y_ps_pool.tile([128, HD + 1], F32, tag="y_ps", name="y_ps") for _ in range(NT_Q)]
                    for b in range(NT_K):
                        lo = max(0, b - 4)
                        hi = min(NT_Q - 1, b)
                        w = (hi - lo + 1) * 128
                        s_ps = s_ps_pool.tile([128, w], F32, tag="s_ps")
                        nc.tensor.matmul(
                            s_ps,
                            kT[g][:, bass.ts(b, 128)],
                            qT[h][:, lo * 128 : lo * 128 + w],
                            start=True,
                            stop=True,
                        )
                        probs = probs_pool.tile([128, w], BF16, tag="probs")
                        nc.scalar.activation(probs, s_ps, AF.Exp, scale=0.125)
                        if b <= 3:
                            # window edge at sub-block t==b (last): keep j <= i
                            nc.gpsimd.affine_select(
                                out=probs[:, (b - lo) * 128 : (b - lo) * 128 + 128],
                                in_=probs[:, (b - lo) * 128 : (b - lo) * 128 + 128],
                                compare_op=mybir.AluOpType.is_ge,
                                fill=0.0,
                                base=0,
                                pattern=[[-1, 128]],
                                channel_multiplier=1,
                            )
                        else:
                            # causal edge at sub-block t==b-4 (first): keep j >= i
                            nc.gpsimd.affine_select(
                                out=probs[:, 0:128],
                                in_=probs[:, 0:128],
                                compare_op=mybir.AluOpType.is_ge,
                                fill=0.0,
                                base=0,
                                pattern=[[1, 128]],
                                channel_multiplier=-1,
                            )
                        for t in range(lo, hi + 1):
                            nc.tensor.matmul(
                                y_ps[t],
                                probs[:, bass.ts(t - lo, 128)],
                                v_sb[b][:, g, :],
                                start=(b == t),
                                stop=(b == t + 4),
                            )
                    for t in range(NT_Q):
                        recz = yev.tile([128, 1], F32, tag="recz")
                        nc.vector.reciprocal(recz, y_ps[t][:, HD : HD + 1])
                        y_n = yev.tile([128, HD], F32, tag="y_n")
                        nc.vector.tensor_scalar_mul(y_n, y_ps[t][:, 0:HD], recz)
                        ytp = yt_ps_pool.tile([HD, 128], F32, tag="ytp")
                        nc.tensor.transpose(ytp, y_n, ident)
                        nc.vector.tensor_copy(
                            yT[h // 2][bass.ts(h % 2, HD), bass.ts(t, 128)], ytp
                        )

          if 4 in phases:
            # ---------------- phase 4: output projection ----------------------
            with tc.tile_pool(name="wo_in", bufs=1) as wo_in, \
                 tc.tile_pool(name="o_ps", bufs=2, space="PSUM") as o_ps_pool, \
                 tc.tile_pool(name="o_ev", bufs=2) as o_ev:
                wo_sb = [wo_in.tile([128, DIM], F32, tag=f"wo{c}", name=f"wo{c}") for c in range(8)]
                for c in range(8):
                    nc.sync.dma_start(out=wo_sb[c], in_=woT[bass.ts(c, 128), :])
                for t in range(NT_Q):
                    o_ps = o_ps_pool.tile([128, DIM], F32, tag="o_ps")
                    for c in range(8):
                        for half in range(2):
                            nc.tensor.matmul(
                                o_ps[:, bass.ts(half, 512)],
                                yT[c][:, bass.ts(t, 128)],
                                wo_sb[c][:, bass.ts(half, 512)],
                                start=(c == 0),
                                stop=(c == 7),
                            )
                    o_sb = o_ev.tile([128, DIM], F32, tag="o_sb")
                    nc.scalar.copy(o_sb, o_ps)
                    nc.sync.dma_start(out=out[bass.ts(t, 128), :], in_=o_sb)

    return nc


_NC_CACHE = None


def kernel(x, wq, wk, wv, wo, q_norm_w, k_norm_w):
    global _NC_CACHE
    x = np.asarray(x, np.float32)
    wq, wk, wv, wo = (np.asarray(a, np.float32) for a in (wq, wk, wv, wo))
    q_norm_w = np.asarray(q_norm_w, np.float32)
    k_norm_w = np.asarray(k_norm_w, np.float32)

    slopes = _alibi_slopes(N_HEADS)
    wqT = np.ascontiguousarray(wq.T)
    wkT = np.ascontiguousarray(wk.T)
    wvT = np.ascontiguousarray(wv.T)
    woT = np.ascontiguousarray(wo.T)

    in_maps = []
    for core in range(8):
        b, c = core // 4, core % 4
        q0 = CH * c
        xh = np.zeros((HALO, DIM), np.float32)
        lo = q0 - WINDOW
        src_lo = max(0, lo)
        xh[src_lo - lo :, :] = x[b, src_lo : q0 + CH, :]
        posk = np.arange(lo, q0 + CH, dtype=np.float32)
        if lo < 0:
            posk[: -lo] = PAD_POS
        posq = np.arange(q0, q0 + CH, dtype=np.float32)
        kaug = np.stack([8.0 * posk, 8.0 * np.ones(HALO, np.float32)])
        qaug = np.stack(
            [
                np.broadcast_to(slopes[:, None], (N_HEADS, CH)),
                -slopes[:, None] * posq[None, :],
            ],
            axis=1,
        ).astype(np.float32)
        in_maps.append(
            {
                "xT": np.ascontiguousarray(xh.T),
                "wqT": wqT,
                "wkT": wkT,
                "wvT": wvT,
                "woT": woT,
                "qaug": np.ascontiguousarray(qaug),
                "kaug": np.ascontiguousarray(kaug),
                "qnw": q_norm_w,
                "knw": k_norm_w,
            }
        )

    if _NC_CACHE is None:
        _NC_CACHE = build_program()
    res = run_bass_kernel_spmd(_NC_CACHE, in_maps, list(range(8)))
    y = np.empty((B, S, DIM), np.float32)
    for core in range(8):
        b, c = core // 4, core % 4
        y[b, CH * c : CH * (c + 1), :] = res.results[core]["out"]
    return y



# revision 19
# speedup vs baseline: 1.7015x; 1.7015x over previous
"""Trainium2 Bass kernel for nn_CodecAttention (GQA + qk-RMSNorm + ALiBi
sliding-window attention + output projection), sharded over 8 NeuronCores as
batch(2) x sequence-chunk(4).

Per core: 512 query tokens (all 16 heads), 1024 halo tokens for K/V.
ALiBi bias + position masking are folded into the score matmul via two extra
contraction rows (rank-2 trick); softmax is the no-max-subtraction variant
(safe because the folded bias is <= 0 and exp(s_kk') ~ e^{+-5}).
Softmax denominator comes from a ones-column appended to V in the PV matmul.
"""

import math
import sys

import numpy as np

sys.path.insert(0, "/opt/trn_rl_repo")

import bass_rust
import concourse.bass as bass
import concourse.tile as tile
from concourse import mybir
from concourse.bass_utils import run_bass_kernel_spmd
from concourse.masks import make_identity
from concourse.vector_clock import ScopedClock

F32 = mybir.dt.float32
F32R = mybir.dt.float32r
BF16 = mybir.dt.bfloat16
AF = mybir.ActivationFunctionType


def _r(ap):
    """Reinterpret an fp32 AP as float32r (TF32-like fast matmul path)."""
    return ap.bitcast(F32R)

DIM = 1024
N_HEADS = 16
N_KV = 4
HD = 64
WINDOW = 512
EPS = 1e-6
B, S = 2, 2048
CH = 512           # query chunk per core
HALO = 1024        # key rows per core (window + chunk)
NT_Q = CH // 128   # 4 query tiles
NT_K = HALO // 128 # 8 key tiles
PAD_POS = -1.0e5

# ---------------------------------------------------------------------------
# Workaround: walrus rejects the TileContext kernel-tail drain when it carries
# more than ~2 sem waits ("Too many sync wait commands").  Spread the
# global-clock waits across multiple SP drain instructions (1 wait each).
_MAXW = 1


def _patched_drain_and_barrier(self, tick_clock, wait_clock):
    nc = self.nc
    probe = nc.sync.drain()
    wait_clock.add_sem_waits(probe.ins, ScopedClock({None: tick_clock.global_clock}))
    si = probe.ins.sync_info
    waits = list(si.on_wait) if si is not None else []
    if len(waits) > _MAXW:
        si.on_wait = waits[:_MAXW]
        for i in range(_MAXW, len(waits), _MAXW):
            d = nc.sync.drain()
            if d.ins.sync_info is None:
                d.ins.sync_info = bass_rust.SyncInfo(
                    on_wait=list(waits[i : i + _MAXW]), on_update=[]
                )
            else:
                d.ins.sync_info.on_wait = list(waits[i : i + _MAXW])
    nc.all_engine_barrier()
    assert self.sems is not None
    popped = nc._tile_sem_poison_stack.pop()
    assert popped is self._sem_poison
    nc.clear_and_free_semaphores(list(self.sems.allocated().values()))
    nc.all_engine_barrier()


tile.TileContext._drain_and_barrier = _patched_drain_and_barrier

# Also split waits on regular instructions: walrus rejects >2 sync waits per
# instruction.  Hook the final-emission path and move excess sem-ge waits onto
# same-engine carrier nops inserted immediately before the instruction.
_SPLIT_MAXW = 1
_orig_add_instruction = tile.TileContext._add_instruction


def _patched_add_instruction(self, inst):
    si = getattr(inst, "sync_info", None)
    if si is not None and si.on_wait and len(si.on_wait) > _SPLIT_MAXW:
        waits = list(si.on_wait)
        eq = [w for w in waits if w.wait_mode != "sem-ge-imm"]
        ge = [w for w in waits if w.wait_mode == "sem-ge-imm"]
        ordered = eq + ge
        keep, extra = ordered[:_SPLIT_MAXW], ordered[_SPLIT_MAXW:]
        assert len(eq) <= _SPLIT_MAXW, "cannot split non-monotonic waits"
        si.on_wait = keep
        for i in range(0, len(extra), _SPLIT_MAXW):
            nop = mybir.InstNoOp(
                name=f"{inst.name}-wsplit{i}",
                sync_info=mybir.SyncInfo(
                    on_wait=list(extra[i : i + _SPLIT_MAXW]), on_update=[]
                ),
                bass_nofuse=True,
                engine=inst.engine,
            )
            _orig_add_instruction(self, nop)
    _orig_add_instruction(self, inst)


tile.TileContext._add_instruction = _patched_add_instruction
# ---------------------------------------------------------------------------


def _alibi_slopes(n_heads):
    r = 2.0 ** (-8.0 / n_heads)
    return np.array([r ** (i + 1) for i in range(n_heads)], dtype=np.float32)


def build_program(repeat=1, phases=(1, 3, 4)):
    nc = bass.Bass("TRN2", target_bir_lowering=False, debug=False, num_devices=8)

    xT = nc.dram_tensor("xT", [DIM, HALO], F32, kind="ExternalInput").ap()
    wqT = nc.dram_tensor("wqT", [DIM, DIM], F32, kind="ExternalInput").ap()
    wkT = nc.dram_tensor("wkT", [DIM, N_KV * HD], F32, kind="ExternalInput").ap()
    wvT = nc.dram_tensor("wvT", [DIM, N_KV * HD], F32, kind="ExternalInput").ap()
    woT = nc.dram_tensor("woT", [DIM, DIM], F32, kind="ExternalInput").ap()
    qaug = nc.dram_tensor("qaug", [N_HEADS, 2, CH], F32, kind="ExternalInput").ap()
    kaug = nc.dram_tensor("kaug", [2, HALO], F32, kind="ExternalInput").ap()
    qnw = nc.dram_tensor("qnw", [DIM], F32, kind="ExternalInput").ap()
    knw = nc.dram_tensor("knw", [N_KV * HD], F32, kind="ExternalInput").ap()
    out = nc.dram_tensor("out", [CH, DIM], F32, kind="ExternalOutput").ap()

    from contextlib import ExitStack
    with tile.TileContext(nc) as tc, ExitStack() as stack:
        # ---------------- persistent pools -------------------------------
        singles = stack.enter_context(tc.tile_pool(name="singles", bufs=1))
        ident = singles.tile([128, 128], F32)
        make_identity(nc, ident)

        qnw_sb = singles.tile([128, 8], F32)
        nc.gpsimd.dma_start(out=qnw_sb, in_=qnw.rearrange("(a b) -> b a", b=128))
        knw_sb = singles.tile([128, 2], F32)
        eps_sb = singles.tile([128, 1], F32)
        nc.vector.memset(eps_sb, EPS)
        nc.gpsimd.dma_start(out=knw_sb, in_=knw.rearrange("(a b) -> b a", b=128))

        # normalized, d-major q/k with 2 aug rows appended (partitions 0..65)
        qT_pool = stack.enter_context(tc.tile_pool(name="qT", bufs=1))
        qT = [qT_pool.tile([66, CH], F32, tag=f"qT{h}", name=f"qT{h}") for h in range(N_HEADS)]
        kT = [qT_pool.tile([66, HALO], F32, tag=f"kT{g}", name=f"kT{g}") for g in range(N_KV)]
        for h in range(N_HEADS):
            nc.gpsimd.dma_start(out=_r(qT[h][64:66, :]), in_=_r(qaug[h, :, :]))
        for g in range(N_KV):
            nc.gpsimd.dma_start(out=_r(kT[g][64:66, :]), in_=_r(kaug[:, :]))

        # token-major V (+ ones column) per key tile: [128, N_KV, HD+1]
        v_pool = stack.enter_context(tc.tile_pool(name="v", bufs=1))
        v_sb = [v_pool.tile([128, N_KV, HD + 1], BF16, tag=f"v{b}", name=f"v{b}") for b in range(NT_K)]
        for b in range(NT_K):
            nc.vector.memset(v_sb[b][:, :, HD : HD + 1], 1.0)

        # ---------------- phase 3: attention -----------------------------
        yT_pool = stack.enter_context(tc.tile_pool(name="yT", bufs=1))
        yT = [yT_pool.tile([128, CH], F32, tag=f"yT{c}", name=f"yT{c}") for c in range(8)]

        for _rep in range(repeat):
          if 1 in phases:
            # ---------------- phase 1+2: projections, norms, transposes ------
            with tc.tile_pool(name="w_in", bufs=1) as w_in, \
                 tc.tile_pool(name="x_in", bufs=16) as x_in, \
                 tc.tile_pool(name="proj_ps", bufs=2, space="PSUM") as proj_ps, \
                 tc.tile_pool(name="tr_ps", bufs=2, space="PSUM") as tr_ps, \
                 tc.tile_pool(name="ev", bufs=2) as ev, \
                 tc.tile_pool(name="scr", bufs=2) as scr:

                wq_sb = [w_in.tile([128, DIM], F32, tag=f"wq{k}", name=f"wq{k}") for k in range(8)]
                wk_sb = [w_in.tile([128, N_KV * HD], F32, tag=f"wk{k}", name=f"wk{k}") for k in range(8)]
                wv_sb = [w_in.tile([128, N_KV * HD], F32, tag=f"wv{k}", name=f"wv{k}") for k in range(8)]
                for k in range(8):
                    nc.sync.dma_start(out=_r(wq_sb[k]), in_=_r(wqT[bass.ts(k, 128), :]))
                    nc.gpsimd.dma_start(out=_r(wk_sb[k]), in_=_r(wkT[bass.ts(k, 128), :]))
                    nc.gpsimd.dma_start(out=_r(wv_sb[k]), in_=_r(wvT[bass.ts(k, 128), :]))

                for tt in range(NT_K):
                    xt = [x_in.tile([128, 128], F32, tag="xt", name="xt") for _ in range(8)]
                    for k in range(8):
                        nc.gpsimd.dma_start(
                            out=_r(xt[k]), in_=_r(xT[bass.ts(k, 128), bass.ts(tt, 128)])
                        )
                    is_q = tt >= NT_K - NT_Q  # last 4 token tiles are the queries
                    k_ps = proj_ps.tile([128, N_KV * HD], F32, tag="k_ps")
                    v_ps = proj_ps.tile([128, N_KV * HD], F32, tag="v_ps")
                    if is_q:
                        q_ps = proj_ps.tile([128, DIM], F32, tag="q_ps", bufs=1)
                    for k in range(8):
                        st, sp = k == 0, k == 7
                        nc.tensor.matmul(k_ps, _r(xt[k]), _r(wk_sb[k]), start=st, stop=sp)
                        nc.tensor.matmul(v_ps, _r(xt[k]), _r(wv_sb[k]), start=st, stop=sp)
                        if is_q:
                            for half in range(2):
                                nc.tensor.matmul(
                                    q_ps[:, bass.ts(half, 512)],
                                    _r(xt[k]),
                                    _r(wq_sb[k][:, bass.ts(half, 512)]),
                                    start=st,
                                    stop=sp,
                                )

                    # V: copy to SBUF (strided into [128, g, 0:64])
                    nc.vector.tensor_copy(
                        v_sb[tt][:, :, 0:HD],
                        v_ps.rearrange("p (g d) -> p g d", g=N_KV),
                    )

                    # K: sum of squares -> rms -> normalize -> transpose
                    ssk = ev.tile([128, 1], F32, tag="ssk")
                    trash_k = scr.tile([128, N_KV * HD], F32, tag="trash_k")
                    nc.scalar.activation(trash_k, k_ps, AF.Square, accum_out=ssk)
                    rmsk = ev.tile([128, 1], F32, tag="rmsk")
                    nc.scalar.activation(rmsk, ssk, AF.Sqrt, scale=1.0 / (N_KV * HD), bias=eps_sb)
                    invk = ev.tile([128, 1], F32, tag="invk")
                    nc.vector.reciprocal(invk, rmsk)
                    k_n = ev.tile([128, N_KV * HD], F32, tag="k_n")
                    nc.vector.tensor_scalar_mul(k_n, k_ps, invk)
                    for c in range(2):  # two 128-dim column blocks = 2 kv heads each
                        ktp = tr_ps.tile([128, 128], F32, tag="tp", name="ktp")
                        nc.tensor.transpose(ktp, k_n[:, bass.ts(c, 128)], ident)
                        for s in range(2):
                            g = 2 * c + s
                            nc.vector.tensor_scalar_mul(
                                _r(kT[g][0:HD, bass.ts(tt, 128)]),
                                ktp[bass.ts(s, HD), :],
                                knw_sb[bass.ts(s, HD), c : c + 1],
                            )

                    if is_q:
                        t = tt - (NT_K - NT_Q)
                        ssq = ev.tile([128, 1], F32, tag="ssq")
                        trash_q = scr.tile([128, DIM], F32, tag="trash_q")
                        nc.scalar.activation(trash_q, q_ps, AF.Square, accum_out=ssq)
                        rmsq = ev.tile([128, 1], F32, tag="rmsq")
                        nc.scalar.activation(rmsq, ssq, AF.Sqrt, scale=1.0 / DIM, bias=eps_sb)
                        invq = ev.tile([128, 1], F32, tag="invq")
                        nc.vector.reciprocal(invq, rmsq)
                        q_n = ev.tile([128, DIM], F32, tag="q_n")
                        nc.vector.tensor_scalar_mul(q_n, q_ps, invq)
                        for c in range(8):  # 128-dim blocks = 2 heads each
                            qtp = tr_ps.tile([128, 128], F32, tag="tp", name="qtp")
                            nc.tensor.transpose(qtp, q_n[:, bass.ts(c, 128)], ident)
                            for s in range(2):
                                h = 2 * c + s
                                nc.vector.tensor_scalar_mul(
                                    _r(qT[h][0:HD, bass.ts(t, 128)]),
                                    qtp[bass.ts(s, HD), :],
                                    qnw_sb[bass.ts(s, HD), c : c + 1],
                                )

          if 3 in phases:
            # ---------------- phase 3: attention -----------------------------

            with tc.tile_pool(name="s_ps", bufs=2, space="PSUM") as s_ps_pool, \
                 tc.tile_pool(name="y_ps", bufs=4, space="PSUM") as y_ps_pool, \
                 tc.tile_pool(name="yt_ps", bufs=1, space="PSUM") as yt_ps_pool, \
                 tc.tile_pool(name="probs", bufs=3) as probs_pool, \
                 tc.tile_pool(name="yev", bufs=2) as yev:

                for h in range(N_HEADS):
                    g = h // 4
                    y_ps = [y_ps_pool.tile([128, HD + 1], F32, tag="y_ps", name="y_ps") for _ in range(NT_Q)]
                    for b in range(NT_K):
                        lo = max(0, b - 4)
                        hi = min(NT_Q - 1, b)
                        w = (hi - lo + 1) * 128
                        s_ps = s_ps_pool.tile([128, w], F32, tag="s_ps")
                        nc.tensor.matmul(
                            s_ps,
                            _r(kT[g][:, bass.ts(b, 128)]),
                            _r(qT[h][:, lo * 128 : lo * 128 + w]),
                            start=True,
                            stop=True,
                        )
                        probs = probs_pool.tile([128, w], BF16, tag="probs")
                        nc.scalar.activation(probs, s_ps, AF.Exp, scale=0.125)
                        if b <= 3:
                            # window edge at sub-block t==b (last): keep j <= i
                            nc.gpsimd.affine_select(
                                out=probs[:, (b - lo) * 128 : (b - lo) * 128 + 128],
                                in_=probs[:, (b - lo) * 128 : (b - lo) * 128 + 128],
                                compare_op=mybir.AluOpType.is_ge,
                                fill=0.0,
                                base=0,
                                pattern=[[-1, 128]],
                                channel_multiplier=1,
                            )
                        else:
                            # causal edge at sub-block t==b-4 (first): keep j >= i
                            nc.gpsimd.affine_select(
                                out=probs[:, 0:128],
                                in_=probs[:, 0:128],
                                compare_op=mybir.AluOpType.is_ge,
                                fill=0.0,
                                base=0,
                                pattern=[[1, 128]],
                                channel_multiplier=-1,
                            )
                        for t in range(lo, hi + 1):
                            nc.tensor.matmul(
                                y_ps[t],
                                probs[:, bass.ts(t - lo, 128)],
                                v_sb[b][:, g, :],
                                start=(b == t),
                                stop=(b == t + 4),
                            )
                    for t in range(NT_Q):
                        recz = yev.tile([128, 1], F32, tag="recz")
                        nc.vector.reciprocal(recz, y_ps[t][:, HD : HD + 1])
                        y_n = yev.tile([128, HD], F32, tag="y_n")
                        nc.vector.tensor_scalar_mul(y_n, y_ps[t][:, 0:HD], recz)
                        ytp = yt_ps_pool.tile([HD, 128], F32, tag="ytp")
                        nc.tensor.transpose(ytp, y_n, ident)
                        nc.vector.tensor_copy(
                            _r(yT[h // 2][bass.ts(h % 2, HD), bass.ts(t, 128)]), ytp
                        )

          if 4 in phases:
            # ---------------- phase 4: output projection ----------------------
            with tc.tile_pool(name="wo_in", bufs=1) as wo_in, \
                 tc.tile_pool(name="o_ps", bufs=2, space="PSUM") as o_ps_pool, \
                 tc.tile_pool(name="o_ev", bufs=2) as o_ev:
                wo_sb = [wo_in.tile([128, DIM], F32, tag=f"wo{c}", name=f"wo{c}") for c in range(8)]
                for c in range(8):
                    nc.sync.dma_start(out=_r(wo_sb[c]), in_=_r(woT[bass.ts(c, 128), :]))
                for t in range(NT_Q):
                    o_ps = o_ps_pool.tile([128, DIM], F32, tag="o_ps")
                    for c in range(8):
                        for half in range(2):
                            nc.tensor.matmul(
                                o_ps[:, bass.ts(half, 512)],
                                _r(yT[c][:, bass.ts(t, 128)]),
                                _r(wo_sb[c][:, bass.ts(half, 512)]),
                                start=(c == 0),
                                stop=(c == 7),
                            )
                    o_sb = o_ev.tile([128, DIM], F32, tag="o_sb")
                    nc.scalar.copy(o_sb, o_ps)
                    nc.sync.dma_start(out=out[bass.ts(t, 128), :], in_=o_sb)

    return nc


_NC_CACHE = None


def kernel(x, wq, wk, wv, wo, q_norm_w, k_norm_w):
    global _NC_CACHE
    x = np.asarray(x, np.float32)
    wq, wk, wv, wo = (np.asarray(a, np.float32) for a in (wq, wk, wv, wo))
    q_norm_w = np.asarray(q_norm_w, np.float32)
    k_norm_w = np.asarray(k_norm_w, np.float32)

    slopes = _alibi_slopes(N_HEADS)
    wqT = np.ascontiguousarray(wq.T)
    wkT = np.ascontiguousarray(wk.T)
    wvT = np.ascontiguousarray(wv.T)
    woT = np.ascontiguousarray(wo.T)

    in_maps = []
    for core in range(8):
        b, c = core // 4, core % 4
        q0 = CH * c
        xh = np.zeros((HALO, DIM), np.float32)
        lo = q0 - WINDOW
        src_lo = max(0, lo)
        xh[src_lo - lo :, :] = x[b, src_lo : q0 + CH, :]
        # Rebase positions to the chunk start so |pos| <= 512: keeps the
        # aug-row values exactly representable at float32r's reduced
        # mantissa (only position DIFFERENCES enter the bias).
        posk = np.arange(lo - q0, CH, dtype=np.float32)
        if lo < 0:
            posk[: -lo] = PAD_POS
        posq = np.arange(0, CH, dtype=np.float32)
        kaug = np.stack([8.0 * posk, 8.0 * np.ones(HALO, np.float32)])
        qaug = np.stack(
            [
                np.broadcast_to(slopes[:, None], (N_HEADS, CH)),
                -slopes[:, None] * posq[None, :],
            ],
            axis=1,
        ).astype(np.float32)
        in_maps.append(
            {
                "xT": np.ascontiguousarray(xh.T),
                "wqT": wqT,
                "wkT": wkT,
                "wvT": wvT,
                "woT": woT,
                "qaug": np.ascontiguousarray(qaug),
                "kaug": np.ascontiguousarray(kaug),
                "qnw": q_norm_w,
                "knw": k_norm_w,
            }
        )

    if _NC_CACHE is None:
        _NC_CACHE = build_program()
    res = run_bass_kernel_spmd(_NC_CACHE, in_maps, list(range(8)))
    y = np.empty((B, S, DIM), np.float32)
    for core in range(8):
        b, c = core // 4, core % 4
        y[b, CH * c : CH * (c + 1), :] = res.results[core]["out"]
    return y

